# revision 7
# baseline (speedup 1.0000x reference)
"""DiT self-attention Bass/Tile kernel for 8 Trainium2 NeuronCores.

Sharding: tensor-parallel over heads. Each of the 8 cores owns 2 of the 16
heads (a 128-wide slice of the hidden dim). To minimize host<->device traffic
over the PJRT/axon tunnel (the dominant cost), each core receives ONE packed
bf16 tensor holding its 512-token slice of the activations plus its weight
slices, already in on-chip layout; the full activation matrix is assembled
on-device with an AllGather over NeuronLink. After the row-sharded output
projection, the 8 partial outputs are summed with an on-device ReduceScatter
so each core returns only its disjoint 512-row slice of the final output.

The attention core (scores/softmax/ctx accumulate) stays fp32r with fp32
PSUM; projections run in bf16 with fp32 accumulation. V is produced directly
in token-major layout (128-token matmul tiles), so no PE transpose and no
identity matrix is needed; the V bias is folded in after softmax
normalization (sum_k p_k (v_k + bv) == (sum_k p_k v_k) + bv).

Shapes are hardcoded for hidden_states [2, 2048, 1024], 16 heads, head dim 64.
"""
import numpy as np
import ml_dtypes

import jax
for _k, _v in (("jax_compilation_cache_dir", "/tmp/jax_pjrt_cache"),
               ("jax_persistent_cache_min_compile_time_secs", 0),
               ("jax_persistent_cache_min_entry_size_bytes", 0)):
    try:
        jax.config.update(_k, _v)
    except Exception:
        pass

import concourse.bass as bass
import concourse.tile as tile
from concourse import bacc, mybir
from concourse.bass_utils import run_bass_kernel_spmd

F32 = mybir.dt.float32
F32R = mybir.dt.float32r
BF16 = mybir.dt.bfloat16
NPBF16 = ml_dtypes.bfloat16

B = 2
S = 2048
H = 1024
NS = B * S          # 4096 rows total
D = 128             # per-core hidden slice (2 heads)
HD = 64             # head dim
SB = 512            # sequence block for projections / attention q-blocks
NSB = NS // SB      # 8 (== n_cores; AllGather block == s-block)
NCHUNK = H // 128   # 8 contraction chunks for projections
NJ = NS // 128      # 32 key chunks globally
NC = 8              # cores
EXP = mybir.ActivationFunctionType.Exp
RG = [[0, 1, 2, 3, 4, 5, 6, 7]]

# packed input column offsets (all bf16, laid out [128, 8192] per core)
HS_OFF = 0          # [128, 8*512]  activations, (p, chunk, s) layout
WQ_OFF = 4096       # [128, 8*128]  Wq slice,    (p, chunk, d) layout
WK_OFF = 5120
WV_OFF = 6144
WO_OFF = 7168       # [128, 1024]   Wo slice,    (d_local, outdim) layout
PK_W = 8192

_CACHED = None


def _build():
    nc = bacc.Bacc("TRN2", target_bir_lowering=False, debug=False,
                   num_devices=NC)

    pk = nc.dram_tensor("pk", [128, PK_W], BF16, kind="ExternalInput").ap()
    bias = nc.dram_tensor("bias", [128, 4], F32, kind="ExternalInput").ap()
    # per-core external output: rows [512c, 512c+512) of the final projection
    out_s = nc.dram_tensor("out_s", [SB, H], BF16, kind="ExternalOutput").ap()

    # internal DRAM for the collectives
    hsl = nc.dram_tensor("hsl", [128, NCHUNK * SB], BF16)   # CC can't read IO
    hsg = nc.dram_tensor("hsg", [NC, 128, NCHUNK * SB], BF16,
                         addr_space="Shared")
    po = nc.dram_tensor("po", [NS, H], BF16)    # partial out-proj (this core)
    rso = nc.dram_tensor("rso", [SB, H], BF16)  # reduce-scattered slice

    with tile.TileContext(nc) as tc:
        with tc.tile_pool(name="singles", bufs=1) as sg:
            # gather the sequence-sharded activations on-device
            nc.sync.dma_start(out=hsl.ap(), in_=pk[:, HS_OFF:HS_OFF + 4096])
            nc.gpsimd.collective_compute(
                "AllGather", mybir.AluOpType.bypass, replica_groups=RG,
                ins=[hsl.ap().opt()], outs=[hsg.ap().opt()])

            # persistent tensors
            qt = sg.tile([128, NS], F32R, tag="qt")
            kt = sg.tile([128, NS], F32R, tag="kt")
            va = sg.tile([128, NJ, 128], F32R, tag="va")
            vb = sg.tile([128, NJ, 128], F32R, tag="vb")
            ctxa = sg.tile([64, NS], BF16, tag="ctxa")
            ctxb = sg.tile([64, NS], BF16, tag="ctxb")
            cstack = sg.tile([128, NS], BF16, tag="cstack")
            wq_sb = sg.tile([128, NCHUNK, D], BF16, tag="wq")
            wk_sb = sg.tile([128, NCHUNK, D], BF16, tag="wk")
            wv_sb = sg.tile([128, NCHUNK, D], BF16, tag="wv")
            wo_sb = sg.tile([128, H], BF16, tag="wo")
            bias_sb = sg.tile([128, 4], F32, tag="bias")

            nc.sync.dma_start(
                out=wq_sb,
                in_=pk[:, WQ_OFF:WQ_OFF + 1024].rearrange("p (c d) -> p c d", c=NCHUNK))
            nc.sync.dma_start(
                out=wk_sb,
                in_=pk[:, WK_OFF:WK_OFF + 1024].rearrange("p (c d) -> p c d", c=NCHUNK))
            nc.sync.dma_start(
                out=wv_sb,
                in_=pk[:, WV_OFF:WV_OFF + 1024].rearrange("p (c d) -> p c d", c=NCHUNK))
            nc.sync.dma_start(out=wo_sb, in_=pk[:, WO_OFF:WO_OFF + 1024])
            nc.sync.dma_start(out=bias_sb, in_=bias)

            # ones columns of va/vb give sum(exp) via the same ctx matmul
            ones_t = sg.tile([128, 64], F32, tag="ones")
            nc.vector.memset(ones_t, 1.0)
            for j in range(NJ):
                nc.vector.tensor_copy(va[:, j, 64:128], ones_t)
                nc.vector.tensor_copy(vb[:, j, 64:128], ones_t)

            # ---------------- phase 1: Q/K projections + token-major V ------
            psu = tc.tile_pool(name="psu", bufs=1, space="PSUM")
            ps1 = ps2 = ps3 = psu.__enter__()
            with tc.tile_pool(name="p1sb", bufs=1) as p1sb:
                for sb in range(NSB):
                    hs = p1sb.tile([128, NCHUNK, SB], BF16, tag="hs", bufs=2)
                    nc.sync.dma_start(
                        out=hs,
                        in_=hsg.ap()[sb].rearrange("p (c s) -> p c s", c=NCHUNK))
                    for wsb, bcol, dest in ((wq_sb, 0, qt), (wk_sb, 1, kt)):
                        pp = ps1.tile([128, SB], F32, tag="misc", bufs=2)
                        for cth in range(NCHUNK):
                            nc.tensor.matmul(pp, lhsT=wsb[:, cth, :],
                                             rhs=hs[:, cth, :],
                                             start=(cth == 0),
                                             stop=(cth == NCHUNK - 1))
                        nc.vector.tensor_scalar_add(
                            dest[:, sb * SB:(sb + 1) * SB], pp,
                            bias_sb[:, bcol:bcol + 1])
                    # V directly in token-major layout, 128 tokens at a time
                    for jj in range(4):
                        j = sb * 4 + jj
                        pv = ps1.tile([128, SB], F32, tag="misc", bufs=2,
                                      name=f"pv{j}")
                        for cth in range(NCHUNK):
                            nc.tensor.matmul(
                                pv[:, 0:128],
                                lhsT=hs[:, cth, jj * 128:(jj + 1) * 128],
                                rhs=wv_sb[:, cth, :],
                                start=(cth == 0), stop=(cth == NCHUNK - 1))
                        nc.vector.tensor_copy(va[:, j, 0:64], pv[:, 0:64])
                        nc.vector.tensor_copy(vb[:, j, 0:64], pv[:, 64:128])

            # ---------------- phase 2+3: attention, assemble, out-proj ------
            with tc.tile_pool(name="p2sb", bufs=1) as p2sb:
                for b in range(B):
                    bcol = b * S
                    QW = 1024
                    for qb in range(S // QW):
                        for hh in range(2):
                            part = slice(hh * 64, hh * 64 + 64)
                            vsel = va if hh == 0 else vb
                            ctxd = ctxa if hh == 0 else ctxb
                            bvs = bias_sb[0:64, 2 + hh:3 + hh]
                            qcols = slice(bcol + qb * QW, bcol + (qb + 1) * QW)
                            cp = ps2.tile([128, QW], F32, tag="ctx", bufs=1)
                            for cc in range(16):
                                kcols = slice(bcol + cc * 128, bcol + (cc + 1) * 128)
                                sp = ps2.tile([128, QW], F32, tag="s", bufs=2)
                                for qh in range(QW // SB):
                                    nc.tensor.matmul(
                                        sp[:, qh * SB:(qh + 1) * SB],
                                        lhsT=kt[part, kcols],
                                        rhs=qt[part, bcol + qb * QW + qh * SB:
                                               bcol + qb * QW + (qh + 1) * SB],
                                        start=True, stop=True)
                                et = p2sb.tile([128, QW], F32R, tag="e", bufs=4)
                                nc.scalar.activation(out=et, in_=sp, func=EXP,
                                                     scale=0.125)
                                for qh in range(QW // SB):
                                    nc.tensor.matmul(
                                        cp[:, qh * SB:(qh + 1) * SB],
                                        lhsT=vsel[:, b * 16 + cc, :],
                                        rhs=et[:, qh * SB:(qh + 1) * SB],
                                        start=(cc == 0), stop=(cc == 15))
                            # rows 0:64 = ctx^T, rows 64:128 = sumexp replicated
                            rc = p2sb.tile([128, QW], F32, tag="rc", bufs=2)
                            nc.vector.reciprocal(rc[64:128, :], cp[64:128, :])
                            rlo = p2sb.tile([64, QW], F32, tag="rlo", bufs=2)
                            nc.sync.dma_start(out=rlo, in_=rc[64:128, :])
                            nc.vector.tensor_mul(ctxd[:, qcols], cp[0:64, :], rlo)
                            nc.vector.tensor_scalar_add(
                                ctxd[:, qcols], ctxd[:, qcols], bvs)
                        # assemble these 1024 ctx columns into cstack
                        for ci in range(2):
                            bc = slice(bcol + qb * QW + ci * SB,
                                       bcol + qb * QW + (ci + 1) * SB)
                            nc.sync.dma_start(out=cstack[0:64, bc], in_=ctxa[:, bc])
                            nc.sync.dma_start(out=cstack[64:128, bc],
                                              in_=ctxb[:, bc])
                        # partial output projection for these 8 q-chunks
                        for qc in range(b * 16 + qb * 8, b * 16 + (qb + 1) * 8):
                            for nb in range(2):
                                op = ps3.tile([128, SB], F32, tag="misc", bufs=2)
                                nc.tensor.matmul(
                                    op, lhsT=cstack[:, qc * 128:(qc + 1) * 128],
                                    rhs=wo_sb[:, nb * SB:(nb + 1) * SB],
                                    start=True, stop=True)
                                ot = p2sb.tile([128, SB], BF16, tag="ot", bufs=3)
                                nc.vector.tensor_copy(ot, op)
                                nc.sync.dma_start(
                                    out=po.ap()[qc * 128:(qc + 1) * 128,
                                                nb * SB:(nb + 1) * SB],
                                    in_=ot)

            psu.__exit__(None, None, None)

            # sum the 8 partial projections; core c keeps rows [512c, 512c+512)
            nc.gpsimd.collective_compute(
                "ReduceScatter", mybir.AluOpType.add, replica_groups=RG,
                ins=[po.ap().opt()], outs=[rso.ap().opt()])
            nc.sync.dma_start(out=out_s, in_=rso.ap())
    nc.compile()
    return nc


def _get_program():
    global _CACHED
    if _CACHED is None:
        _CACHED = _build()
    return _CACHED


def _pack_w(w):
    # [1024, 128] (h, d) -> [128, 8*128] (p, chunk, d) on-chip layout
    return np.ascontiguousarray(
        w.reshape(NCHUNK, 128, D).transpose(1, 0, 2).reshape(128, NCHUNK * D)
    ).astype(NPBF16)


def kernel(hidden_states, Wq, bq, Wk, bk, Wv, bv, Wo, bo):
    nc = _get_program()
    hs = np.asarray(hidden_states, dtype=np.float32).reshape(NS, H)
    # [8 cores, 128 p, 8 chunk, 512 s] on-chip layout of each token slice
    hs_lay = np.ascontiguousarray(
        hs.reshape(NC, SB, NCHUNK, 128).transpose(0, 3, 2, 1)).astype(NPBF16)
    Wq = np.asarray(Wq, dtype=np.float32)
    Wk = np.asarray(Wk, dtype=np.float32)
    Wv = np.asarray(Wv, dtype=np.float32)
    Wo = np.asarray(Wo, dtype=np.float32)
    bqf = np.asarray(bq, dtype=np.float32)
    bkf = np.asarray(bk, dtype=np.float32)
    bvf = np.asarray(bv, dtype=np.float32)

    in_maps = []
    for c in range(NC):
        r = slice(D * c, D * (c + 1))
        pk = np.empty((128, PK_W), dtype=NPBF16)
        pk[:, HS_OFF:HS_OFF + 4096] = hs_lay[c].reshape(128, 4096)
        pk[:, WQ_OFF:WQ_OFF + 1024] = _pack_w(Wq[r].T)
        pk[:, WK_OFF:WK_OFF + 1024] = _pack_w(Wk[r].T)
        pk[:, WV_OFF:WV_OFF + 1024] = _pack_w(Wv[r].T)
        pk[:, WO_OFF:WO_OFF + 1024] = Wo[:, r].T.astype(NPBF16)
        bias = np.zeros((128, 4), dtype=np.float32)
        bias[:, 0] = bqf[r]
        bias[:, 1] = bkf[r]
        bias[0:64, 2] = bvf[r][0:64]
        bias[0:64, 3] = bvf[r][64:128]
        in_maps.append({"pk": pk, "bias": bias})

    res = run_bass_kernel_spmd(nc, in_maps, list(range(NC)))
    out = np.concatenate(
        [res.results[c]["out_s"].astype(np.float32) for c in range(NC)], axis=0)
    out += np.asarray(bo, dtype=np.float32)
    return out.reshape(B, S, H)
